# revision 1
# baseline (speedup 1.0000x reference)
"""Conv2d 3x3 VALID kernel for Trainium2, batch-sharded across 8 NeuronCores.

Problem: input [32,128,64,64] f32, weights [256,128,3,3] f32 ->
output [32,256,62,62] f32 (stride 1, no padding).

Strategy (per core, 4 images):
  - Cin=128 == SBUF partition dim == matmul contraction dim.
  - Input image b lives in SBUF as [128, 4096] (row-major h*64+w).
  - out[y, x] = sum_{kh,kw,ci} in[ci, (y+kh)*64 + x+kw] * W[co,ci,kh,kw].
    For a block of 8 output rows and tap (kh,kw), the rhs is the strided AP
    in_sb[:, (y0+kh)*64+kw :][8 rows step 64, 62 cols step 1] -> N=496
    moving columns, accumulated over the 9 taps into one PSUM bank.
  - Cout=256 -> two halves of 128 (PSUM partition limit).
  - Weights are DMA'd raw [co,(ci kh kw)] and transposed on-chip with PE
    transposes into lhsT layout [ci, tap*256 + half*128 + co].
  - matmuls run as float32r (fp32 bits, 1 cycle/row at N>=256). The walrus
    birverifier requires every producer feeding an FP32r matmul to emit
    FP32r-typed output, hence the bitcasts on the DMAs/copies.
"""

import numpy as np

import concourse.bass as bass
import concourse.mybir as mybir
import concourse.tile as tile
from concourse import bacc
from concourse.bass_utils import run_bass_kernel_spmd
from concourse.masks import make_identity

F32 = mybir.dt.float32
F32R = mybir.dt.float32r

B, CIN, H, W = 32, 128, 64, 64
COUT, KH, KW = 256, 3, 3
OH, OW = H - KH + 1, W - KW + 1  # 62, 62
N_CORES = 8
BL = B // N_CORES  # 4 images per core

IMG_STRIDE = H * W  # 4096
W_FREE = CIN * KH * KW  # 1152
N_TAPS = KH * KW  # 9
ROWS_PER_CHUNK = 8  # 8 output rows x 62 cols = 496 <= 512 (one PSUM bank)


def _conv_body(nc, tc, out_d, x_d, w_d, use_f32r=True):
    mm_dt = F32R if use_f32r else F32
    x_r = x_d.rearrange("b c h w -> b c (h w)")  # [BL, 128, 4096]
    w_r = w_d.rearrange("co ci kh kw -> co (ci kh kw)")  # [256, 1152]

    with (
        tc.tile_pool(name="const", bufs=1) as cpool,
        tc.tile_pool(name="psum", bufs=8, space=bass.MemorySpace.PSUM) as psum_pool,
        tc.tile_pool(name="outp", bufs=4) as out_pool,
    ):
        in_sb = cpool.tile([128, BL * IMG_STRIDE], F32)
        w_raw = cpool.tile([128, 2 * W_FREE], F32)
        w_l = cpool.tile([128, N_TAPS * COUT], F32)  # [ci, t*256 + h*128 + co]
        ident = cpool.tile([128, 128], F32)

        make_identity(nc, ident)

        # Weights first (longest dependency chain: DMA -> transpose -> copy).
        # One instruction: dma_start issue costs ~610ns on the sync
        # sequencer, so batch; the HW DGE stripes rows across all 16 queues.
        nc.sync.dma_start(
            out=w_raw.rearrange("p (h c) -> p h c", h=2),
            in_=w_r.rearrange("(h p) c -> p h c", h=2),
        )
        # Image 0 next (needed by the first conv matmuls) in two pieces so
        # its first rows land early; then the remaining images whole.
        for b in range(BL):
            for c0, c1 in ([(0, 2048), (2048, 4096)] if b == 0 else [(0, 4096)]):
                nc.sync.dma_start(
                    out=in_sb[
                        :, b * IMG_STRIDE + c0 : b * IMG_STRIDE + c1
                    ].bitcast(mm_dt),
                    in_=x_r[b][:, c0:c1].bitcast(mm_dt),
                )

        # Transpose weights: w_raw half h viewed as [co, (ci t)] -> per tap
        # [co, ci] (ci at stride 9) -> PE transpose -> [ci, co].
        for h in range(2):
            w_v = w_raw[:, h * W_FREE : (h + 1) * W_FREE].rearrange(
                "p (ci t) -> p t ci", t=N_TAPS
            )
            for t in range(N_TAPS):
                ps = psum_pool.tile([128, 512], F32, tag="ps")
                nc.tensor.transpose(ps[:, :128], w_v[:, t, :], ident)
                nc.vector.tensor_copy(
                    w_l[:, t * COUT + h * 128 : t * COUT + h * 128 + 128].bitcast(
                        mm_dt
                    ),
                    ps[:, :128],
                )

        # Main loop: 2 halves x BL images x 8 row-blocks x 9 taps.
        for h in range(2):
            for b in range(BL):
                img_v = in_sb[
                    :, b * IMG_STRIDE : (b + 1) * IMG_STRIDE
                ].rearrange("p (r x) -> p r x", x=W)  # [128, 64, 64]
                for y0 in range(0, OH, ROWS_PER_CHUNK):
                    nrows = min(ROWS_PER_CHUNK, OH - y0)
                    size = nrows * OW
                    ps = psum_pool.tile([128, 512], F32, tag="ps")
                    ps_v = ps[:, :size].rearrange("p (r x) -> p r x", x=OW)
                    for t in range(N_TAPS):
                        kh, kw = divmod(t, KW)
                        lhsT = w_l[:, t * COUT + h * 128 : t * COUT + h * 128 + 128]
                        # rhs: rectangular window, nrows stride-64 rows x 62 cols
                        rhs = img_v[:, y0 + kh : y0 + kh + nrows, kw : kw + OW]
                        if use_f32r:
                            lhsT = lhsT.bitcast(F32R)
                            rhs = rhs.bitcast(F32R)
                        nc.tensor.matmul(
                            ps_v,
                            lhsT,
                            rhs,
                            start=(t == 0),
                            stop=(t == N_TAPS - 1),
                        )
                    ot = out_pool.tile([128, ROWS_PER_CHUNK * OW], F32)
                    nc.vector.tensor_copy(ot[:, :size], ps[:, :size])
                    nc.sync.dma_start(
                        out=out_d[b, h * 128 : (h + 1) * 128, y0 : y0 + nrows, :],
                        in_=ot[:, :size].rearrange("p (r x) -> p r x", x=OW),
                    )


def build_module(use_f32r=True):
    nc = bacc.Bacc(
        "TRN2", target_bir_lowering=False, debug=False, num_devices=N_CORES
    )
    x_d = nc.dram_tensor(
        "input_image", [BL, CIN, H, W], F32, kind="ExternalInput"
    ).ap()
    w_d = nc.dram_tensor("weights", [COUT, CIN, KH, KW], F32, kind="ExternalInput").ap()
    out_d = nc.dram_tensor("out", [BL, COUT, OH, OW], F32, kind="ExternalOutput").ap()
    with tile.TileContext(nc) as tc:
        _conv_body(nc, tc, out_d, x_d, w_d, use_f32r=use_f32r)
    nc.compile()
    return nc


_NC_CACHE = {}


def _get_module(use_f32r=True):
    key = use_f32r
    if key not in _NC_CACHE:
        _NC_CACHE[key] = build_module(use_f32r=use_f32r)
    return _NC_CACHE[key]


def kernel(input_image: np.ndarray, weights: np.ndarray) -> np.ndarray:
    input_image = np.ascontiguousarray(input_image, dtype=np.float32)
    weights = np.ascontiguousarray(weights, dtype=np.float32)
    nc = _get_module()
    in_maps = [
        {
            "input_image": input_image[i * BL : (i + 1) * BL],
            "weights": weights,
        }
        for i in range(N_CORES)
    ]
    res = run_bass_kernel_spmd(nc, in_maps, list(range(N_CORES))).results
    return np.concatenate([r["out"] for r in res], axis=0)



# revision 3
# speedup vs baseline: 1.2241x; 1.2241x over previous
"""Conv2d 3x3 VALID kernel for Trainium2, batch-sharded across 8 NeuronCores.

Problem: input [32,128,64,64] f32, weights [256,128,3,3] f32 ->
output [32,256,62,62] f32 (stride 1, no padding).

Strategy (per core, 4 images):
  - Cin=128 == SBUF partition dim == matmul contraction dim.
  - Input image b lives in SBUF as [128, 4096] (row-major h*64+w), converted
    once to bf16 (rel-err budget 2e-2; bf16 matmul error ~6e-3).
  - out[y, x] = sum_{kh,kw,ci} in[ci, (y+kh)*64 + x+kw] * W[co,ci,kh,kw].
  - Loop order: for (half, image): TAP-OUTER over 8 PSUM banks.
    For tap t, all 8 row-chunks (8 rows x 62 cols = 496 <= 512) stream with
    the SAME stationary weights, so the PE amortizes LDWEIGHTS 8x (and bf16
    enables Fast Weight Load, halving each LDW vs fp32).
  - Cout=256 -> two halves of 128 (PSUM partition limit).
  - Weights are DMA'd raw [co,(ci kh kw)], PE-transposed per tap to
    lhsT layout [ci, tap*256 + half*128 + co], stored bf16.
  - PSUM->SBUF drains run on the Scalar (ACT) engine, which is fast at PSUM;
    fp32->bf16 input conversion runs on the Vector engine. Both overlap MMs.
"""

import numpy as np

import concourse.bass as bass
import concourse.mybir as mybir
import concourse.tile as tile
from concourse import bacc
from concourse.bass_utils import run_bass_kernel_spmd
from concourse.masks import make_identity

F32 = mybir.dt.float32
BF16 = mybir.dt.bfloat16

B, CIN, H, W = 32, 128, 64, 64
COUT, KH, KW = 256, 3, 3
OH, OW = H - KH + 1, W - KW + 1  # 62, 62
N_CORES = 8
BL = B // N_CORES  # 4 images per core

IMG_STRIDE = H * W  # 4096
W_FREE = CIN * KH * KW  # 1152
N_TAPS = KH * KW  # 9
ROWS_PER_CHUNK = 8  # 8 output rows x 62 cols = 496 <= 512 (one PSUM bank)
N_CHUNKS = (OH + ROWS_PER_CHUNK - 1) // ROWS_PER_CHUNK  # 8


def _conv_body(nc, tc, out_d, x_d, w_d):
    x_r = x_d.rearrange("b c h w -> b c (h w)")  # [BL, 128, 4096]
    w_r = w_d.rearrange("co ci kh kw -> co (ci kh kw)")  # [256, 1152]

    with (
        tc.tile_pool(name="const", bufs=1) as cpool,
        tc.tile_pool(name="psum", bufs=8, space=bass.MemorySpace.PSUM) as psum_pool,
        tc.tile_pool(name="outp", bufs=4) as out_pool,
    ):
        in_f32 = cpool.tile([128, BL * IMG_STRIDE], F32)
        in_bf = cpool.tile([128, BL * IMG_STRIDE], BF16)
        w_raw = cpool.tile([128, 2 * W_FREE], F32)
        w_l = cpool.tile([128, N_TAPS * COUT], BF16)  # [ci, t*256 + h*128 + co]
        ident = cpool.tile([128, 128], F32)

        make_identity(nc, ident)

        # Weights first (longest dependency chain: DMA -> transpose -> copy).
        nc.sync.dma_start(
            out=w_raw.rearrange("p (h c) -> p h c", h=2),
            in_=w_r.rearrange("(h p) c -> p h c", h=2),
        )
        # Image 0 next (needed by the first conv matmuls) in two pieces so
        # its first rows land early; then the remaining images whole.
        for b in range(BL):
            for c0, c1 in ([(0, 2048), (2048, 4096)] if b == 0 else [(0, 4096)]):
                nc.sync.dma_start(
                    out=in_f32[:, b * IMG_STRIDE + c0 : b * IMG_STRIDE + c1],
                    in_=x_r[b][:, c0:c1],
                )
                nc.vector.tensor_copy(
                    in_bf[:, b * IMG_STRIDE + c0 : b * IMG_STRIDE + c1],
                    in_f32[:, b * IMG_STRIDE + c0 : b * IMG_STRIDE + c1],
                )

        # Transpose weights: w_raw half h viewed as [co, (ci t)] -> per tap
        # [co, ci] (ci at stride 9) -> PE transpose -> [ci, co] -> bf16.
        for h in range(2):
            w_v = w_raw[:, h * W_FREE : (h + 1) * W_FREE].rearrange(
                "p (ci t) -> p t ci", t=N_TAPS
            )
            for t in range(N_TAPS):
                ps = psum_pool.tile([128, 512], F32, tag="ps")
                nc.tensor.transpose(ps[:, :128], w_v[:, t, :], ident)
                nc.vector.tensor_copy(
                    w_l[:, t * COUT + h * 128 : t * COUT + h * 128 + 128],
                    ps[:, :128],
                )

        # Main loop: 2 halves x BL images, tap-outer over 8 PSUM banks.
        for h in range(2):
            for b in range(BL):
                img_v = in_bf[
                    :, b * IMG_STRIDE : (b + 1) * IMG_STRIDE
                ].rearrange("p (r x) -> p r x", x=W)  # [128, 64, 64]
                ps_list = []
                for c in range(N_CHUNKS):
                    ps_c = psum_pool.tile([128, 512], F32, tag="ps", name=f"ps_{c}")
                    ps_list.append(ps_c)
                for t in range(N_TAPS):
                    kh, kw = divmod(t, KW)
                    lhsT = w_l[:, t * COUT + h * 128 : t * COUT + h * 128 + 128]
                    for c in range(N_CHUNKS):
                        y0 = c * ROWS_PER_CHUNK
                        nrows = min(ROWS_PER_CHUNK, OH - y0)
                        size = nrows * OW
                        ps_v = ps_list[c][:, :size].rearrange(
                            "p (r x) -> p r x", x=OW
                        )
                        rhs = img_v[:, y0 + kh : y0 + kh + nrows, kw : kw + OW]
                        nc.tensor.matmul(
                            ps_v,
                            lhsT,
                            rhs,
                            start=(t == 0),
                            stop=(t == N_TAPS - 1),
                        )
                for c in range(N_CHUNKS):
                    y0 = c * ROWS_PER_CHUNK
                    nrows = min(ROWS_PER_CHUNK, OH - y0)
                    size = nrows * OW
                    ot = out_pool.tile([128, ROWS_PER_CHUNK * OW], F32)
                    nc.scalar.copy(ot[:, :size], ps_list[c][:, :size])
                    nc.sync.dma_start(
                        out=out_d[b, h * 128 : (h + 1) * 128, y0 : y0 + nrows, :],
                        in_=ot[:, :size].rearrange("p (r x) -> p r x", x=OW),
                    )


def build_module():
    nc = bacc.Bacc(
        "TRN2", target_bir_lowering=False, debug=False, num_devices=N_CORES
    )
    x_d = nc.dram_tensor(
        "input_image", [BL, CIN, H, W], F32, kind="ExternalInput"
    ).ap()
    w_d = nc.dram_tensor("weights", [COUT, CIN, KH, KW], F32, kind="ExternalInput").ap()
    out_d = nc.dram_tensor("out", [BL, COUT, OH, OW], F32, kind="ExternalOutput").ap()
    with tile.TileContext(nc) as tc:
        _conv_body(nc, tc, out_d, x_d, w_d)
    nc.compile()
    return nc


_NC_CACHE = {}


def _get_module():
    if "m" not in _NC_CACHE:
        _NC_CACHE["m"] = build_module()
    return _NC_CACHE["m"]


def kernel(input_image: np.ndarray, weights: np.ndarray) -> np.ndarray:
    input_image = np.ascontiguousarray(input_image, dtype=np.float32)
    weights = np.ascontiguousarray(weights, dtype=np.float32)
    nc = _get_module()
    in_maps = [
        {
            "input_image": input_image[i * BL : (i + 1) * BL],
            "weights": weights,
        }
        for i in range(N_CORES)
    ]
    res = run_bass_kernel_spmd(nc, in_maps, list(range(N_CORES))).results
    return np.concatenate([r["out"] for r in res], axis=0)
